# revision 58
# baseline (speedup 1.0000x reference)
"""Swin-style windowed-attention block on 8 TRN2 NeuronCores (data-parallel over batch).

Self-contained: host-side prep (fold norms/scale into weights, bias+mask tables,
bf16 casts, transposes, zero-pad windows 49->64 tokens) + a fused Bass/Tile kernel.

Per-core layout: tokens are processed in window PAIRS; each window is padded to
64 tokens so a pair fills 128 partitions exactly (win0 @ rows 0:49, win1 @ rows
64:113, pad rows zero/garbage and never stored).

v2: full-pair attention — one [83-contraction, 128, 128] QK matmul per
(pair, head) with augmented rows carrying bias, per-window masks and
cross-window masks; exp over the whole pair; one [128,128]x[128,33] AV per
head reading v directly from the v-GEMM drain (no scatter DMAs).
"""

import sys

sys.path.insert(0, "/opt/trn_rl_repo")

import numpy as np
import ml_dtypes

import concourse.bass as bass
import concourse.bacc as bacc
import concourse.tile as tile
import concourse.mybir as mybir
from concourse.bass_utils import run_bass_kernel_spmd

BF16 = ml_dtypes.bfloat16
FP8 = ml_dtypes.float8_e4m3
FP32 = mybir.dt.float32
BF16_DT = mybir.dt.bfloat16
FP8_DT = mybir.dt.float8e4
W8SCALE = 64.0

# ---- static geometry ----
WH, WW = 7, 7
S = 49                     # valid tokens per window
SP = 64                    # padded tokens per window
C = 256                    # channels
NH = 8                     # heads
HD = 32                    # head dim
NWIN = 256                 # windows per batch image
B = 8                      # batch == number of cores
GRID = 16                  # 16x16 window grid
SCALE = HD ** -0.5
EPS = 1e-5
MASK_VAL = -30000.0

NPAIR = NWIN // 2          # 128 window pairs per core
PPC = 8                    # pairs per chunk
NCHUNK = NPAIR // PPC      # 16 chunks
TPP = 2 * SP               # 128 padded tokens per pair
VPP = 2 * S                # 98 valid tokens per pair
TPC = PPC * TPP            # 1024 padded tokens per chunk
VPC = PPC * VPP            # 784 valid tokens per chunk
NTOK = NWIN * S            # 12544 valid tokens per core
NTOKP = NPAIR * TPP        # 16384 padded tokens per core
SLAB = 4 * TPP             # 512-wide moving slab (4 pairs)
SLABS = TPC // SLAB        # 2 slabs per chunk
KAUG = 32 + S + 2          # 83 contraction rows for augmented QK

ActF = mybir.ActivationFunctionType
Alu = mybir.AluOpType


# --------------------------------------------------------------------------
# host-side preparation
# --------------------------------------------------------------------------

def _relative_position_index():
    ch, cw = np.arange(WH), np.arange(WW)
    coords = np.stack(np.meshgrid(ch, cw, indexing="ij")).reshape(2, -1)
    rel = coords[:, :, None] - coords[:, None, :]
    rel = rel.transpose(1, 2, 0).astype(np.int64)
    rel[..., 0] += WH - 1
    rel[..., 1] += WW - 1
    rel[..., 0] *= 2 * WW - 1
    return rel.sum(-1)                                    # (S, S)


def _window_mask_types():
    """Per-window mask type: 0 none, 1 bottom-row, 2 right-col, 3 corner."""
    h = w = GRID
    s1, s2 = WH - WH // 2, WW - WW // 2
    m = np.zeros((h, w, WH, WW, WH, WW), dtype=bool)
    m[-1, :, :s1, :, s1:, :] = True
    m[-1, :, s1:, :, :s1, :] = True
    m[:, -1, :, :s2, :, s2:] = True
    m[:, -1, :, s2:, :, :s2] = True
    m = m.reshape(h * w, S, S)
    types = np.zeros(NWIN, dtype=np.int64)
    rr, cc = np.divmod(np.arange(NWIN), GRID)
    types[(rr == GRID - 1) & (cc < GRID - 1)] = 1
    types[(rr < GRID - 1) & (cc == GRID - 1)] = 2
    types[(rr == GRID - 1) & (cc == GRID - 1)] = 3
    masks = np.zeros((4, S, S), dtype=np.float32)
    masks[1] = np.where(m[GRID * (GRID - 1)], MASK_VAL, 0.0)
    masks[2] = np.where(m[GRID - 1], MASK_VAL, 0.0)
    masks[3] = np.where(m[NWIN - 1], MASK_VAL, 0.0)
    return types, masks


def _pair_types():
    types, _ = _window_mask_types()
    combos = []
    ptype = np.zeros(NPAIR, dtype=np.int64)
    for j in range(NPAIR):
        c = (int(types[2 * j]), int(types[2 * j + 1]))
        if c not in combos:
            combos.append(c)
        ptype[j] = combos.index(c)
    assert len(combos) <= 4, combos
    while len(combos) < 4:
        combos.append((0, 0))
    return ptype, combos


_PTYPE, _PCOMBOS = _pair_types()


def _tile_kxoc(wT):
    """[K, OC] -> [128, K//128, OC] with K = 128*kt + p."""
    K, OC = wT.shape
    return np.ascontiguousarray(wT.reshape(K // 128, 128, OC).transpose(1, 0, 2))


def host_prep(inputs):
    x = np.asarray(inputs["x"], dtype=np.float32)          # (B, N, S, C)
    qkv_w = np.asarray(inputs["qkv_w"], dtype=np.float32)
    qkv_b = np.asarray(inputs["qkv_b"], dtype=np.float32)
    proj_w = np.asarray(inputs["proj_w"], dtype=np.float32)
    proj_b = np.asarray(inputs["proj_b"], dtype=np.float32)
    n1g = np.asarray(inputs["norm1_g"], dtype=np.float32)
    n1b = np.asarray(inputs["norm1_b"], dtype=np.float32)
    n2g = np.asarray(inputs["norm2_g"], dtype=np.float32)
    n2b = np.asarray(inputs["norm2_b"], dtype=np.float32)
    w1 = np.asarray(inputs["mlp_w1"], dtype=np.float32)
    b1 = np.asarray(inputs["mlp_b1"], dtype=np.float32)
    w2 = np.asarray(inputs["mlp_w2"], dtype=np.float32)
    b2 = np.asarray(inputs["mlp_b2"], dtype=np.float32)
    table = np.asarray(inputs["bias_table"], dtype=np.float32)

    # fold layernorm affine into the following matmuls
    qkv_w_f = qkv_w * n1g[None, :]
    qkv_b_f = qkv_b + qkv_w @ n1b
    w1_f = w1 * n2g[None, :]
    b1_f = b1 + w1 @ n2b

    wq = qkv_w_f[0:C] * SCALE
    bq = qkv_b_f[0:C] * SCALE
    wk = qkv_w_f[C:2 * C]
    bk = qkv_b_f[C:2 * C]
    wv = qkv_w_f[2 * C:3 * C]
    bv = qkv_b_f[2 * C:3 * C]

    common = {
        "wq": _tile_kxoc(wq.T).astype(BF16),
        "wk": _tile_kxoc(wk.T).astype(BF16),
        "wv": _tile_kxoc(wv.T).astype(BF16),
        "wp": _tile_kxoc(proj_w.T).astype(BF16),
        "w1": _tile_kxoc(w1_f.T * W8SCALE).astype(FP8),
        "w2": _tile_kxoc(w2.T * W8SCALE).astype(FP8),
        "bq": np.ascontiguousarray(bq.reshape(2, 128).T).astype(np.float32),
        "bk": np.ascontiguousarray(bk.reshape(2, 128).T).astype(np.float32),
        "b1": np.ascontiguousarray(b1_f.reshape(8, 128).T).astype(np.float32),
    }

    # augmented-K rows 32:81: k'[32+i, h, (pt), t] = (bias_h + mask_pt)[i, t]
    # (i indexes the query within its window; q' carries identity rows so the
    # matmul adds bias[s%64, t%64]).  Padded-t columns get MASK_VAL so exp()
    # zeroes padded key rows.
    rel = _relative_position_index()
    bias_sht = table[rel].transpose(2, 0, 1)               # [h, s, t]
    _, masks = _window_mask_types()                        # [4, s, t]
    bmps = np.full((S, 4, NH, 2, SP), MASK_VAL, dtype=np.float32)
    for pt, (t0, t1) in enumerate(_PCOMBOS):
        for h in range(NH):
            bmps[:, pt, h, 0, 0:S] = bias_sht[h] + masks[t0]
            bmps[:, pt, h, 1, 0:S] = bias_sht[h] + masks[t1]
    common["bmps"] = bmps.reshape(S, 4, NH, TPP).astype(BF16)
    # q identity rows (shared by both windows of the pair)
    qid = np.zeros((S, 2, SP), dtype=np.float32)
    for r in range(2):
        qid[:, r, 0:S] = np.eye(S, dtype=np.float32)
    common["qid"] = qid.reshape(S, TPP).astype(BF16)
    # cross-window mask rows 81:83
    kxr = np.zeros((2, TPP), dtype=np.float32)
    kxr[0, SP:] = MASK_VAL          # q in win0, k in win1
    kxr[1, :SP] = MASK_VAL          # q in win1, k in win0
    qxr = np.zeros((2, TPP), dtype=np.float32)
    qxr[0, :SP] = 1.0
    qxr[1, SP:] = 1.0
    common["kxr"] = kxr.astype(BF16)
    common["qxr"] = qxr.astype(BF16)

    extra = {
        "bv_nz": bool(np.any(bv != 0.0)),
        "pb_nz": bool(np.any(proj_b != 0.0)),
        "b2_nz": bool(np.any(b2 != 0.0)),
        "qb_nz": bool(np.any(bq != 0.0)),
        "kb_nz": bool(np.any(bk != 0.0)),
        "b1_nz": bool(np.any(b1_f != 0.0)),
    }
    if not extra["qb_nz"]:
        del common["bq"]
    if not extra["kb_nz"]:
        del common["bk"]
    if not extra["b1_nz"]:
        del common["b1"]
    if extra["bv_nz"]:
        common["bvbc"] = np.tile(bv[None, :], (128, 1)).astype(np.float32)
    if extra["pb_nz"]:
        common["pbbc"] = np.tile(proj_b[None, :], (128, 1)).astype(np.float32)
    if extra["b2_nz"]:
        common["b2bc"] = np.tile(b2[None, :], (128, 1)).astype(np.float32)

    in_maps = []
    for b in range(B):
        m = dict(common)
        xp = np.zeros((NWIN, SP, C), dtype=np.float32)
        xp[:, :S, :] = x[b]
        m["x"] = xp.reshape(NTOKP, C).astype(BF16)
        in_maps.append(m)
    return in_maps, extra


# --------------------------------------------------------------------------
# kernel builder
# --------------------------------------------------------------------------

DEFAULT_CFG = {
    "tr_a": "dma",    # xn transpose: "pe" | "dma"
    "tr_d": "pe",     # attn transpose
    "tr_e": "pe",     # xn2 transpose (fp8 out -> must be pe)
    "rstd": "sqrt",   # "sqrt" | "lnexp"
    "warm_ldw": False,  # dummy LDWEIGHTS to keep the PE HAM warm during stalls
    "ln_apply_gps": True,  # LN apply as 2 big broadcast TT ops on gpsimd
}


def _warm(nc, cst, cfg):
    if cfg.get("warm_ldw"):
        nc.tensor.ldweights(cst["ident"][:, :])


def build_program(n_pairs=NPAIR, bv_nz=False, pb_nz=False, b2_nz=False,
                  qb_nz=False, kb_nz=False, b1_nz=False, cfg=None):
    assert n_pairs % PPC == 0
    n_chunks = n_pairs // PPC
    cfg = dict(DEFAULT_CFG, **(cfg or {}))

    nc = bacc.Bacc("TRN2", target_bir_lowering=False, debug=False)

    ext = {}
    ext["x"] = nc.dram_tensor("x", [n_pairs * TPP, C], BF16_DT, kind="ExternalInput")
    ext["out"] = nc.dram_tensor("out", [n_pairs * VPP, C], FP32, kind="ExternalOutput")
    ext["wq"] = nc.dram_tensor("wq", [128, 2, C], BF16_DT, kind="ExternalInput")
    ext["wk"] = nc.dram_tensor("wk", [128, 2, C], BF16_DT, kind="ExternalInput")
    ext["wv"] = nc.dram_tensor("wv", [128, 2, C], BF16_DT, kind="ExternalInput")
    ext["wp"] = nc.dram_tensor("wp", [128, 2, C], BF16_DT, kind="ExternalInput")
    ext["w1"] = nc.dram_tensor("w1", [128, 2, 4 * C], FP8_DT, kind="ExternalInput")
    ext["w2"] = nc.dram_tensor("w2", [128, 8, C], FP8_DT, kind="ExternalInput")
    ext["bq"] = nc.dram_tensor("bq", [128, 2], FP32, kind="ExternalInput") if qb_nz else None
    ext["bk"] = nc.dram_tensor("bk", [128, 2], FP32, kind="ExternalInput") if kb_nz else None
    ext["b1"] = nc.dram_tensor("b1", [128, 8], FP32, kind="ExternalInput") if b1_nz else None
    ext["bmps"] = nc.dram_tensor("bmps", [S, 4, NH, TPP], BF16_DT, kind="ExternalInput")
    ext["qid"] = nc.dram_tensor("qid", [S, TPP], BF16_DT, kind="ExternalInput")
    ext["kxr"] = nc.dram_tensor("kxr", [2, TPP], BF16_DT, kind="ExternalInput")
    ext["qxr"] = nc.dram_tensor("qxr", [2, TPP], BF16_DT, kind="ExternalInput")
    ext["bvbc"] = nc.dram_tensor("bvbc", [128, C], FP32, kind="ExternalInput") if bv_nz else None
    ext["pbbc"] = nc.dram_tensor("pbbc", [128, C], FP32, kind="ExternalInput") if pb_nz else None
    ext["b2bc"] = nc.dram_tensor("b2bc", [128, C], FP32, kind="ExternalInput") if b2_nz else None

    with tile.TileContext(nc) as tc:
        _body(tc, n_chunks, ext, cfg)

    nc.compile()
    return nc


def _bcast_rows(ap2, sizes):
    """[R, TPP] -> [R, *sizes(bcast 0-stride), TPP] DRAM AP."""
    return bass.AP(tensor=ap2.tensor, offset=ap2.offset,
                   ap=[ap2.ap[0]] + [[0, s] for s in sizes] + [ap2.ap[1]])


def _body(tc, n_chunks, ext, cfg):
    nc = tc.nc
    import contextlib
    with contextlib.ExitStack() as ctx:
        const = ctx.enter_context(tc.tile_pool(name="const", bufs=1))
        cst = {}
        for name, shape, dt in (
            ("wq", [128, 2, C], BF16_DT), ("wk", [128, 2, C], BF16_DT),
            ("wv", [128, 2, C], BF16_DT), ("wp", [128, 2, C], BF16_DT),
            ("w1", [128, 2, 4 * C], FP8_DT), ("w2", [128, 8, C], FP8_DT),
            ("bq", [128, 2], FP32), ("bk", [128, 2], FP32),
            ("b1", [128, 8], FP32),
            ("bvbc", [128, C], FP32), ("pbbc", [128, C], FP32),
            ("b2bc", [128, C], FP32),
        ):
            if ext.get(name) is None:
                cst[name] = None
                continue
            t = const.tile(shape, dt, tag=name)
            nc.sync.dma_start(out=t[:], in_=ext[name].ap())
            cst[name] = t
        ident = const.tile([128, 128], BF16_DT, tag="ident")
        from concourse.masks import make_identity
        make_identity(nc, ident[:])
        cst["ident"] = ident
        eps_sb = const.tile([128, 1], FP32, tag="eps")
        nc.vector.memset(eps_sb[:], EPS)
        cst["eps"] = eps_sb

        pools = {}
        for name, bufs in (("xip", 3), ("xop", 2), ("xnp", 2), ("tp", 2),
                           ("qkp", 2), ("vp2p", 2), ("etp", 2),
                           ("atp", 2), ("x2p", 2), ("hp", 1), ("statp", 2)):
            pools[name] = ctx.enter_context(tc.tile_pool(name=name, bufs=bufs))

        # q'/k' augmented tiles: rows 0:32 head data, 32:81 identity/bias,
        # 81:83 cross-window mask; one pair per chunk-parity.
        qk_aug = []
        for par in range(2):
            qa = const.tile([KAUG, NH, PPC, TPP], BF16_DT, tag=f"qaug{par}")
            ka = const.tile([KAUG, NH, PPC, TPP], BF16_DT, tag=f"kaug{par}")
            nc.sync.dma_start(out=qa[32:32 + S, :, :, :],
                              in_=_bcast_rows(ext["qid"].ap(), [NH * PPC]))
            nc.sync.dma_start(out=qa[32 + S:KAUG, :, :, :],
                              in_=_bcast_rows(ext["qxr"].ap(), [NH * PPC]))
            nc.sync.dma_start(out=ka[32 + S:KAUG, :, :, :],
                              in_=_bcast_rows(ext["kxr"].ap(), [NH * PPC]))
            qk_aug.append((qa, ka))
        kpat_state = [None, None]

        for name, shape, dt, bufs in (
            ("ps_S", [128, NH // 2, TPP], FP32, 3),
            ("ps_A", [128, NH, HD + 1], FP32, 2),
            ("ps_w", [128, SLAB], FP32, 3),
        ):
            pools[name] = ctx.enter_context(
                tc.tile_pool(name=name, bufs=bufs, space="PSUM"))

        # software pipeline (2 deep): emit chunk i+2's input phase A
        # (load+LN1+xpose) and chunk i+1's QKV phase B between chunk i's
        # attention (C, D) and output (E, F) phases so every engine queue has
        # cross-chunk work and the PE never drains.
        states = {0: _phase_a(tc, 0, ext, cst, pools, cfg)}
        if n_chunks > 1:
            states[1] = _phase_a(tc, 1, ext, cst, pools, cfg)
        _phase_b(tc, 0, states[0], ext, cst, pools, qk_aug, kpat_state, cfg)
        for ci in range(n_chunks):
            stc = states.pop(ci)
            _phase_c(tc, ci, stc, cst, pools, qk_aug, cfg)
            _phase_d(tc, ci, stc, cst, pools, cfg)
            if ci + 2 < n_chunks:
                states[ci + 2] = _phase_a(tc, ci + 2, ext, cst, pools, cfg)
            if ci + 1 < n_chunks:
                _phase_b(tc, ci + 1, states[ci + 1], ext, cst, pools, qk_aug,
                         kpat_state, cfg)
            _phase_e(tc, ci, stc, ext, cst, pools, cfg)
            _phase_f(tc, ci, stc, ext, cst, pools, cfg)


def _load_pattern(nc, ext, ka, pattern):
    """Load bias+mask rows 32:81 for the chunk's pair-type pattern."""
    # group consecutive equal pair types into broadcast DMAs
    j = 0
    while j < PPC:
        j2 = j
        while j2 + 1 < PPC and pattern[j2 + 1] == pattern[j]:
            j2 += 1
        for h in range(NH):
            src = ext["bmps"].ap()[:, pattern[j], h, :]    # [S, TPP]
            src_b = bass.AP(tensor=src.tensor, offset=src.offset,
                            ap=[src.ap[0], [0, j2 - j + 1], src.ap[1]])
            nc.sync.dma_start(out=ka[32:32 + S, h, j:j2 + 1, :], in_=src_b)
        j = j2 + 1


def _transpose_pairs(tc, pools, src_t, dst, ident, mode, drain="dve"):
    """Transpose [128, PPC, C] token-major into kt-major [128, 2, PPC, TPP]."""
    nc = tc.nc
    assert mode == "pe"
    for j in range(PPC):
        ps = pools["ps_w"].tile([128, 2, 128], BF16_DT, tag="wide", name="ps_tr")
        for ch in range(2):
            nc.tensor.transpose(ps[:, ch, :], src_t[:, j, 128 * ch:128 * (ch + 1)],
                                ident[:, :])
        if drain == "dve":
            nc.vector.tensor_copy(out=dst[:, :, j, :], in_=ps[:, :, :])
        else:
            nc.scalar.activation(dst[:, :, j, :], ps[:, :, :], ActF.Copy)


def _transpose_pairs_jm(tc, pools, src_t, dst, ident, mode):
    """Like _transpose_pairs but dst is j-major [128, PPC, 2, TPP]."""
    nc = tc.nc
    if mode == "dma":
        for g in range(0, PPC, 4):
            nc.sync.dma_start_transpose(out=dst[:, g:g + 4, :, :],
                                        in_=src_t[:, g:g + 4, :])
        return
    for j in range(PPC):
        ps = pools["ps_w"].tile([128, 2, 128], BF16_DT, tag="wide", name="ps_tr")
        for ch in range(2):
            nc.tensor.transpose(ps[:, ch, :], src_t[:, j, 128 * ch:128 * (ch + 1)],
                                ident[:, :])
        nc.vector.tensor_copy(out=dst[:, j, :, :], in_=ps[:, :, :])


def _layernorm(nc, pools, x_t, xn_t, eps_sb, cfg):
    """x_t [128, PPC, 256] f32 -> xn_t bf16 ((x-mu)*rstd)."""
    statp = pools["statp"]
    mv = statp.tile([128, PPC, 2], FP32, tag="mv")
    for j in range(PPC):
        st = statp.tile([128, 6], FP32, tag="bnst")
        nc.vector.bn_stats(st[:], x_t[:, j, :])
        nc.vector.bn_aggr(mv[:, j, :], st[:])
    rstd = statp.tile([128, PPC], FP32, tag="rstd")
    if cfg["rstd"] == "lnexp":
        # stays in the natural_log_exp table set (if chosen by walrus)
        nc.scalar.activation(rstd[:], mv[:, :, 1], ActF.Ln, bias=eps_sb[:])
        nc.scalar.activation(rstd[:], rstd[:], ActF.Exp, scale=-0.5)
    else:
        nc.scalar.activation(rstd[:], mv[:, :, 1], ActF.Sqrt, bias=eps_sb[:])
        nc.vector.reciprocal(rstd[:], rstd[:])
    if cfg.get("ln_apply_gps"):
        mu_b = bass.AP(tensor=mv.tensor, offset=mv.offset,
                       ap=[mv.ap[0], [2, PPC], [0, C]])
        r_b = bass.AP(tensor=rstd.tensor, offset=rstd.offset,
                      ap=[rstd.ap[0], [1, PPC], [0, C]])
        nc.gpsimd.tensor_sub(xn_t[:], x_t[:], mu_b)
        nc.gpsimd.tensor_mul(xn_t[:], xn_t[:], r_b)
    else:
        for j in range(PPC):
            nc.vector.tensor_scalar(
                out=xn_t[:, j, :], in0=x_t[:, j, :],
                scalar1=mv[:, j, 0:1], scalar2=rstd[:, j:j + 1],
                op0=Alu.subtract, op1=Alu.mult,
            )


def _strided_heads(ap, h):
    """aug[0:32, {h, h+4}, :, :] as a single AP."""
    base = ap[0:32, h, :, :]
    hstride = ap.ap[1][0]
    return bass.AP(tensor=base.tensor, offset=base.offset,
                   ap=[base.ap[0], [4 * hstride, 2]] + list(base.ap[1:]))


def _phase_a(tc, ci, ext, cst, pools, cfg):
    """Load + LN1 + transpose. Returns per-chunk state dict."""
    nc = tc.nc
    st = {"ci": ci}
    x_t = pools["xip"].tile([128, PPC, C], BF16_DT, tag="xi", name="x_t")
    nc.sync.dma_start(
        out=x_t[:],
        in_=ext["x"][ci * TPC:(ci + 1) * TPC, :].rearrange("(j p) c -> p j c", p=TPP),
    )
    xn_t = pools["xnp"].tile([128, PPC, C], BF16_DT, tag="xn")
    _layernorm(nc, pools, x_t, xn_t, cst["eps"], cfg)
    xnT = pools["tp"].tile([128, PPC, 2, TPP], BF16_DT, tag="xnT")
    _transpose_pairs_jm(tc, pools, xn_t, xnT, cst["ident"], cfg["tr_a"])
    st["x_t"], st["xnT"] = x_t, xnT
    return st


def _phase_b(tc, ci, st, ext, cst, pools, qk_aug, kpat_state, cfg):
    """QKV projections into augmented tiles + vp2."""
    nc = tc.nc
    xnT = st["xnT"]
    qT = pools["qkp"].tile([128, 2, PPC, TPP], BF16_DT, tag="qT")
    kT = pools["qkp"].tile([128, 2, PPC, TPP], BF16_DT, tag="kT")
    qa, ka = qk_aug[ci % 2]
    pattern = tuple(int(_PTYPE[ci * PPC + j]) for j in range(PPC))
    if kpat_state[ci % 2] != pattern:
        _load_pattern(nc, ext, ka, pattern)
        kpat_state[ci % 2] = pattern
    for s2 in range(SLABS):
        jsl = slice(4 * s2, 4 * s2 + 4)
        for qk, (dst, wname, bname) in enumerate(
                ((qT, "wq", "bq"), (kT, "wk", "bk"))):
            w_sb, b_sb = cst[wname], cst[bname]
            for octl in range(2):
                ps = pools["ps_w"].tile([128, SLAB], FP32, tag="wide", name="ps_qk")
                for kt in range(2):
                    nc.tensor.matmul(
                        ps[:],
                        lhsT=w_sb[:, kt, 128 * octl:128 * (octl + 1)],
                        rhs=xnT[:, jsl, kt, :],
                        start=(kt == 0), stop=(kt == 1),
                    )
                if b_sb is None:
                    if qk == 0:
                        nc.scalar.activation(dst[:, octl, jsl, :], ps[:], ActF.Copy)
                    else:
                        nc.vector.tensor_copy(out=dst[:, octl, jsl, :], in_=ps[:])
                elif qk == 0:
                    nc.scalar.activation(
                        dst[:, octl, jsl, :], ps[:],
                        ActF.Identity, bias=b_sb[:, octl:octl + 1],
                    )
                else:
                    nc.vector.tensor_scalar_add(
                        dst[:, octl, jsl, :], ps[:],
                        b_sb[:, octl:octl + 1],
                    )
                _warm(nc, cst, cfg)
    # move head rows into the augmented tiles (heads h and h+4 per DMA)
    for dst, srct in ((qa, qT), (ka, kT)):
        for h in range(4):
            nc.sync.dma_start(out=_strided_heads(dst, h),
                              in_=srct[32 * h:32 * h + 32, :, :, :])

    # v: token-major psum -> drain straight into the AV rhs layout
    vp2 = pools["vp2p"].tile([128, PPC, NH, HD + 1], BF16_DT, tag="vp2")
    # ones only at valid key rows: pad-t rows must contribute 0 to the
    # softmax denominator (their et columns are skipped by the strided exp)
    nc.vector.memset(vp2[:, :, :, 0:1], 0.0)
    for r in range(2):
        nc.vector.memset(vp2[SP * r:SP * r + S, :, :, 0:1], 1.0)
    for j in range(PPC):
        ps = pools["ps_w"].tile([128, C], FP32, tag="wide", name="ps_v")
        for kt in range(2):
            nc.tensor.matmul(
                ps[:], lhsT=xnT[:, j, kt, :], rhs=cst["wv"][:, kt, :],
                start=(kt == 0), stop=(kt == 1),
            )
        nc.vector.tensor_copy(
            out=vp2[:, j, :, 1:HD + 1],
            in_=ps[:, :].rearrange("p (h d) -> p h d", h=NH),
        )
        if cst["bvbc"] is not None:
            nc.vector.tensor_add(
                vp2[:, j, :, 1:HD + 1], vp2[:, j, :, 1:HD + 1],
                cst["bvbc"][:, :].rearrange("p (h d) -> p h d", h=NH))
        _warm(nc, cst, cfg)
    st["vp2"] = vp2


def _phase_c(tc, ci, st, cst, pools, qk_aug, cfg):
    """Attention (full pair per matmul)."""
    nc = tc.nc
    qa, ka = qk_aug[ci % 2]
    vp2 = st["vp2"]
    attn_t = pools["atp"].tile([128, PPC, C], BF16_DT, tag="attn")
    for j in range(PPC):
        et = pools["etp"].tile([128, NH, TPP], BF16_DT, tag="et")
        if cfg.get("sim_safe"):
            et_p = et[:, :, :].rearrange("p h (r t) -> p h r t", r=2)
            nc.vector.memset(et_p[:, :, :, S:SP], 0.0)
        for g in range(2):
            pss = pools["ps_S"].tile([128, NH // 2, TPP], FP32, tag="S")
            for hh in range(NH // 2):
                h = 4 * g + hh
                nc.tensor.matmul(
                    pss[:, hh, :],
                    lhsT=ka[:, h, j, :],
                    rhs=qa[:, h, j, :],
                    start=True, stop=True,
                )
            # skip the 15 pad columns per window: et pad cols go stale, but
            # pad-t rows are killed by v=0 + zero ones-col, and pad-s outputs
            # are never stored
            et_v = et[:, 4 * g:4 * g + 4, :].rearrange("p h (r t) -> p h r t", r=2)
            ps_v = pss[:, :, :].rearrange("p h (r t) -> p h r t", r=2)
            nc.scalar.activation(et_v[:, :, :, 0:S], ps_v[:, :, :, 0:S], ActF.Exp)
        psa = pools["ps_A"].tile([128, NH, HD + 1], FP32, tag="A")
        for h in range(NH):
            nc.tensor.matmul(
                psa[:, h, :],
                lhsT=et[:, h, :],
                rhs=vp2[:, j, h, :],
                start=True, stop=True,
            )
        rec = pools["statp"].tile([128, NH], FP32, tag="rec")
        nc.vector.tensor_scalar_max(out=rec[:], in0=psa[:, :, 0], scalar1=1e-30)
        nc.vector.reciprocal(rec[:], rec[:])
        rec_s = rec[:]
        rec_b = bass.AP(tensor=rec_s.tensor, offset=rec_s.offset,
                        ap=list(rec_s.ap) + [[0, HD]])
        nc.vector.tensor_mul(
            attn_t[:, j, :].rearrange("p (h d) -> p h d", h=NH),
            psa[:, :, 1:HD + 1], rec_b,
        )
    st["attn_t"] = attn_t


def _phase_d(tc, ci, st, cst, pools, cfg):
    attnT = pools["tp"].tile([128, PPC, 2, TPP], BF16_DT, tag="attnT")
    _transpose_pairs_jm(tc, pools, st["attn_t"], attnT, cst["ident"], cfg["tr_d"])
    st["attnT"] = attnT


def _phase_e(tc, ci, st, ext, cst, pools, cfg):
    """proj + resid1 + LN2 + transpose."""
    nc = tc.nc
    attnT, x_t = st["attnT"], st["x_t"]
    x2_t = pools["x2p"].tile([128, PPC, C], FP32, tag="x2")
    for j in range(PPC):
        ps = pools["ps_w"].tile([128, C], FP32, tag="wide", name="ps_pj")
        for kt in range(2):
            nc.tensor.matmul(
                ps[:], lhsT=attnT[:, j, kt, :], rhs=cst["wp"][:, kt, :],
                start=(kt == 0), stop=(kt == 1),
            )
        if cst["pbbc"] is not None:
            nc.vector.tensor_add(ps[:], ps[:], cst["pbbc"][:])
        nc.vector.tensor_add(x2_t[:, j, :], ps[:], x_t[:, j, :])
        _warm(nc, cst, cfg)

    xn2_t = pools["xnp"].tile([128, PPC, C], BF16_DT, tag="xn2")
    _layernorm(nc, pools, x2_t, xn2_t, cst["eps"], cfg)
    xn2T = pools["tp"].tile([128, 2, PPC, TPP], FP8_DT, tag="xn2T")
    _transpose_pairs(tc, pools, xn2_t, xn2T, cst["ident"], cfg["tr_e"])
    st["x2_t"], st["xn2T"] = x2_t, xn2T


def _phase_f(tc, ci, st, ext, cst, pools, cfg):
    """MLP + resid2 + store."""
    nc = tc.nc
    xn2T, x2_t = st["xn2T"], st["x2_t"]
    hT = pools["hp"].tile([128, 8, PPC, TPP], FP8_DT, tag="hT")
    for s2 in range(SLABS):
        jsl = slice(4 * s2, 4 * s2 + 4)
        for m in range(8):
            ps = pools["ps_w"].tile([128, SLAB], FP32, tag="wide", name="ps_m1")
            nc.tensor.matmul(
                ps[:], lhsT=cst["w1"][:, :, 128 * m:128 * (m + 1)],
                rhs=xn2T[:, :, jsl, :],
                start=True, stop=True,
                perf_mode=mybir.MatmulPerfMode.DoubleRow,
            )
            nc.scalar.activation(
                hT[:, m, jsl, :], ps[:], ActF.Gelu,
                bias=cst["b1"][:, m:m + 1] if cst["b1"] is not None else 0.0,
                scale=1.0 / W8SCALE,
            )
            _warm(nc, cst, cfg)

    out_t = pools["xop"].tile([128, PPC, C], FP32, tag="xo", name="out_t")
    for j in range(PPC):
        ps = pools["ps_w"].tile([128, C], FP32, tag="wide", name="ps_m2")
        for k2 in range(4):
            nc.tensor.matmul(
                ps[:], lhsT=hT[:, 2 * k2:2 * k2 + 2, j, :],
                rhs=cst["w2"][:, 2 * k2:2 * k2 + 2, :],
                start=(k2 == 0), stop=(k2 == 3),
                perf_mode=mybir.MatmulPerfMode.DoubleRow,
            )
        if cst["b2bc"] is not None:
            nc.vector.tensor_add(ps[:], ps[:], cst["b2bc"][:])
        nc.vector.scalar_tensor_tensor(
            out=out_t[:, j, :], in0=ps[:], scalar=1.0 / W8SCALE,
            in1=x2_t[:, j, :], op0=Alu.mult, op1=Alu.add,
        )
        _warm(nc, cst, cfg)

    # compact output: window (2j+r) valid rows SP*r : SP*r+S
    for r in range(2):
        dst = ext["out"][ci * VPC + r * S:, :]
        dst_ap = bass.AP(
            tensor=dst.tensor, offset=dst.offset,
            ap=[[C, S], [2 * S * C, PPC], [1, C]],
        )
        nc.sync.dma_start(out=dst_ap, in_=out_t[SP * r:SP * r + S, :, :])


# --------------------------------------------------------------------------
# entry point
# --------------------------------------------------------------------------

_CACHE = {}


_FLAG_NAMES = ("bv_nz", "pb_nz", "b2_nz", "qb_nz", "kb_nz", "b1_nz")


def _get_program(key_flags):
    if key_flags not in _CACHE:
        _CACHE[key_flags] = build_program(
            NPAIR, **dict(zip(_FLAG_NAMES, key_flags)),
        )
    return _CACHE[key_flags]


def kernel(**inputs):
    in_maps, extra = host_prep(inputs)
    nc = _get_program(tuple(extra[f] for f in _FLAG_NAMES))
    res = run_bass_kernel_spmd(nc, in_maps, core_ids=list(range(B)))
    out = np.stack([res.results[i]["out"] for i in range(B)], axis=0)
    return out.reshape(B, NWIN, S, C).astype(np.float32)
